# revision 1
# baseline (speedup 1.0000x reference)
"""Dense 2-layer GAT on 8 Trainium2 NeuronCores (Bass/Tile, SPMD).

Problem: B=4 graphs, N=2048 nodes, F_in=128, H=8 heads, F_hid=64, C=32.
Sharding: 2 cores per graph, each core owns 1024 attention rows (r-shard)
for all heads in layer 1 and for layer 2; only a tiny [1024,65] AllGather
of Wh2 crosses cores between the layers.

Key structure (per core, layout [c(node, partitions) x r(row, free)]):
  - masked softmax == adj * exp(leaky_relu(e)); mask applied ADDITIVELY
    before the lrelu via M = (adjT-1)*150 fused into one DVE
    scalar_tensor_tensor: u = (f1rep + f2_pp) + M.
  - lrelu as one stt: t = (u*alpha) max u  (in place).
  - softmax normalizer deferred past the matmul by appending 64 ones
    columns to Wh -> psum rows 64:128 hold the row-sum replicated, so
    reciprocal_approx_fast + one stt do the normalize.
  - f1 (free-axis) produced pre-replicated across partitions via rank-1
    matmuls with host-built (W@a1) x ones weights; f2 (per-partition)
    via xT.T @ (W@a2).
"""
import os
import numpy as np
import ml_dtypes

import concourse.bass as bass
import concourse.mybir as mybir
import concourse.tile as tile
from concourse.bass_utils import run_bass_kernel_spmd
from concourse.vector_clock import VectorClock, ScopedClock

F32 = mybir.dt.float32
F16 = mybir.dt.float16
Alu = mybir.AluOpType
Act = mybir.ActivationFunctionType

B, N, F_IN, H, F_HID, C = 4, 2048, 128, 8, 64, 32
RSH = N // 2           # rows per core
NCB = N // 128         # 16 c-chunks
ALPHA = 0.2
OUT_SLOPE = 0.01
MASK = -150.0
GROUPS = [[0, 1], [2, 3], [4, 5], [6, 7]]

# engine assignment knob: (h*16+cb) % LRELU_GPS_MOD == 0 -> lrelu on gpsimd
LRELU_SCHED = os.environ.get("GAT_LRELU_SCHED", "D")

# ---------------------------------------------------------------------------
# Patch: Tile's kernel-tail drain aggregates one wait per outstanding proc
# lane into a single Drain instruction; walrus codegen only supports one
# sync wait per instruction ("Too many sync wait commands").  Split into a
# chain of single-wait drains instead.
import concourse.tile as _tile_mod

_ORIG_DRAIN = _tile_mod.TileContext._drain_and_barrier


def _split_drain_and_barrier(self, tick_clock, wait_clock):
    vals = eval(repr(tick_clock.global_clock).split("VectorClock(", 1)[1].rstrip(")"))
    for i, v in enumerate(vals):
        if v <= 0:
            continue
        part = VectorClock()
        part.require_at_least(i, v)
        d = self.nc.sync.drain()
        wait_clock.add_sem_waits(d.ins, ScopedClock({None: part}))
    self.nc.sync.drain()
    self.nc.all_engine_barrier()
    popped = self.nc._tile_sem_poison_stack.pop()
    assert popped is self._sem_poison
    self.nc.clear_and_free_semaphores(list(self.sems.allocated().values()))
    self.nc.all_engine_barrier()


_tile_mod.TileContext._drain_and_barrier = _split_drain_and_barrier

def _legalize_multi_waits(nc):
    """Walrus codegen accepts at most one sync wait per instruction; hoist
    extra waits onto preceding same-engine sequencer NOPs."""
    Op = nc.isa.Opcode

    def mk_nop(engine):
        return nc.engines[engine]._isa(Op.NEURON_ISA_TPB_OPCODE_NOP, {})

    n_fix = 0
    for f in nc.m.functions:
        for bb in f.blocks:
            insts = list(bb.instructions)
            if not any(i.sync_info and i.sync_info.on_wait
                       and len(i.sync_info.on_wait) > 1 for i in insts):
                continue
            new = []
            for inst in insts:
                si = inst.sync_info
                if si and si.on_wait and len(si.on_wait) > 1:
                    waits = list(si.on_wait)
                    for w in waits[:-1]:
                        nop = mk_nop(inst.engine)
                        nop.sync_info = mybir.SyncInfo(on_wait=[w], on_update=[])
                        new.append(nop)
                        n_fix += 1
                    inst.sync_info = mybir.SyncInfo(
                        on_wait=[waits[-1]], on_update=list(si.on_update or []))
                new.append(inst)
            bb.instructions = new
    return n_fix
# ---------------------------------------------------------------------------


def build_nc():
    nc = bass.Bass(num_devices=8)

    xT_e = nc.dram_tensor("xT", [F_IN, N], F32, kind="ExternalInput")
    xtr_e = nc.dram_tensor("xtr", [F_IN, RSH], F32, kind="ExternalInput")
    madj_e = nc.dram_tensor("madj", [N, RSH], F16, kind="ExternalInput")
    wall_e = nc.dram_tensor("wall", [F_IN, H * F_HID], F32, kind="ExternalInput")
    wa2_e = nc.dram_tensor("wa2", [F_IN, H], F32, kind="ExternalInput")
    w1rep_e = nc.dram_tensor("w1rep", [F_IN, H * 128], F32, kind="ExternalInput")
    woaug_e = nc.dram_tensor("woaug", [H * F_HID, 65], F16, kind="ExternalInput")
    wo1rep_e = nc.dram_tensor("wo1rep", [H * F_HID, 128], F16, kind="ExternalInput")
    out_e = nc.dram_tensor("out", [C, RSH], F32, kind="ExternalOutput")
    DBG = os.environ.get("GAT_DEBUG", "0") == "1"
    if DBG:
        dbg_names = {}
        for nm, shp, dt in [("dbg_f1rep0", [128, RSH], F16), ("dbg_f2sb", [128, NCB * H], F32),
                            ("dbg_whaug0", [128, H * 128], F16), ("dbg_u00", [128, RSH], F16),
                            ("dbg_p00", [128, RSH], F16), ("dbg_hT0", [128, RSH], F16),
                            ("dbg_ph1_7", [128, RSH], F32), ("dbg_wh2_0", [128, 65], F16),
                            ("dbg_g1rep", [128, RSH], F16), ("dbg_ccsb", [128, 8 * 65], F16),
                            ("dbg_rs0", [128, RSH], F32), ("dbg_hn0", [64, RSH], F16),
                            ("dbg_q0", [64, RSH], F16)]:
            dbg_names[nm] = nc.dram_tensor(nm, shp, dt, kind="ExternalOutput")
    cc_in = nc.dram_tensor("cc_in", [RSH, 65], F16)
    cc_out = nc.dram_tensor("cc_out", [N, 65], F16)

    with tile.TileContext(nc) as tc:
        from contextlib import ExitStack
        with ExitStack() as ctx:
            res = ctx.enter_context(tc.tile_pool(name="res", bufs=1))
            work = ctx.enter_context(tc.tile_pool(name="work", bufs=4))
            ep = ctx.enter_context(tc.tile_pool(name="ep", bufs=2))

            def _lrelu(site, u_ap):
                """leaky-relu in place; engine picked per site to balance
                DVE (bottleneck) against idle GPSIMD and slack ACT."""
                kind = LRELU_SCHED[site % len(LRELU_SCHED)]
                if kind == "G":
                    v = work.tile([128, RSH * 2], F16, tag="v", name=f"v{site}", bufs=2)
                    nc.gpsimd.tensor_scalar(out=v, in0=u_ap, scalar1=ALPHA,
                                            scalar2=0.0, op0=Alu.mult, op1=Alu.bypass)
                    nc.gpsimd.tensor_tensor(out=u_ap, in0=u_ap, in1=v, op=Alu.max)
                elif kind == "A":
                    nc.scalar.activation(out=u_ap, in_=u_ap, func=Act.Lrelu,
                                         alpha=ALPHA)
                else:
                    nc.vector.scalar_tensor_tensor(out=u_ap, in0=u_ap, scalar=ALPHA,
                                                   in1=u_ap, op0=Alu.mult, op1=Alu.max)

            # ---------------- input loads ----------------
            xT = res.tile([F_IN, N], F32, tag="xT")
            nc.sync.dma_start(out=xT, in_=xT_e[:, :])
            xtr = res.tile([F_IN, RSH], F32, tag="xtr")
            nc.sync.dma_start(out=xtr, in_=xtr_e[:, :])
            wall = res.tile([F_IN, H * F_HID], F32, tag="wall")
            nc.sync.dma_start(out=wall, in_=wall_e[:, :])
            wa2 = res.tile([F_IN, H], F32, tag="wa2")
            nc.sync.dma_start(out=wa2, in_=wa2_e[:, :])
            w1rep = res.tile([F_IN, H * 128], F32, tag="w1rep")
            nc.sync.dma_start(out=w1rep, in_=w1rep_e[:, :])
            woaug = [res.tile([128, 65], F16, tag=f"woaug{k}", name=f"woaug{k}") for k in range(4)]
            wo1rep = [res.tile([128, 128], F16, tag=f"wo1rep{k}", name=f"wo1rep{k}") for k in range(4)]
            for k in range(4):
                nc.sync.dma_start(out=woaug[k], in_=woaug_e[k * 128:(k + 1) * 128, :])
                nc.sync.dma_start(out=wo1rep[k], in_=wo1rep_e[k * 128:(k + 1) * 128, :])
            madj = []
            for cb in range(NCB):
                t = res.tile([128, RSH], F16, tag=f"madj{cb}", name=f"madj{cb}")
                nc.sync.dma_start(out=t, in_=madj_e[cb * 128:(cb + 1) * 128, :])
                madj.append(t)

            whaug = [res.tile([128, H * 128], F16, tag=f"whaug{cb}", name=f"whaug{cb}") for cb in range(NCB)]
            f1rep = [res.tile([128, RSH], F16, tag=f"f1rep{h}", name=f"f1rep{h}") for h in range(H)]
            f2sb = res.tile([128, NCB * H], F32, tag="f2sb")
            hT = [res.tile([128, RSH], F16, tag=f"hT{k}", name=f"hT{k}") for k in range(4)]

            with tc.tile_pool(name="ps_set", bufs=2, space="PSUM") as ps_set:
                # Wh per c-chunk: [128, 512] = all heads side by side
                for cb in range(NCB):
                    pwh = ps_set.tile([128, H * F_HID], F32, tag="set_a")
                    nc.tensor.matmul(pwh, lhsT=xT[:, cb * 128:(cb + 1) * 128],
                                     rhs=wall, start=True, stop=True)
                    # strided copy into whaug (64 Wh cols of each 128-col head block)
                    wh_v = whaug[cb].rearrange("p (hh q) -> p hh q", q=128)
                    dst = wh_v[:, :, 0:F_HID]
                    src = pwh.rearrange("p (hh o) -> p hh o", o=F_HID)
                    if cb % 2 == 0:
                        nc.vector.tensor_copy(out=dst, in_=src)
                    else:
                        nc.scalar.activation(out=dst, in_=src, func=Act.Copy)
                    nc.vector.memset(wh_v[:, :, F_HID:128], 1.0)

                    # f2 for this chunk: [128, H]
                    pf2 = ps_set.tile([128, H], F32, tag="set_a")
                    nc.tensor.matmul(pf2, lhsT=xT[:, cb * 128:(cb + 1) * 128],
                                     rhs=wa2, start=True, stop=True)
                    nc.vector.tensor_copy(out=f2sb[:, cb * H:(cb + 1) * H], in_=pf2)

                # f1 replicated across partitions: per head [128, 1024]
                for h in range(H):
                    pf1 = ps_set.tile([128, RSH], F32, tag="set_f1")
                    for j in range(2):
                        nc.tensor.matmul(pf1[:, j * 512:(j + 1) * 512],
                                         lhsT=w1rep[:, h * 128:(h + 1) * 128],
                                         rhs=xtr[:, j * 512:(j + 1) * 512],
                                         start=True, stop=True)
                    if h % 2 == 0:
                        nc.vector.tensor_copy(out=f1rep[h], in_=pf1)
                    else:
                        nc.scalar.activation(out=f1rep[h], in_=pf1, func=Act.Copy)

            with tc.tile_pool(name="ps_main", bufs=3, space="PSUM") as ps_main:
                # ---------------- layer 1 ----------------
                for hp in range(H // 2):
                    ha, hb = 2 * hp, 2 * hp + 1
                    ph1s = [ps_main.tile([128, RSH], F32, tag="h1", name=f"ph1_{h}")
                            for h in (ha, hb)]
                    for cb in range(NCB):
                        u = work.tile([128, RSH * 2], F16, tag="u")
                        for i, h in enumerate((ha, hb)):
                            nc.vector.scalar_tensor_tensor(
                                out=u[:, i * RSH:(i + 1) * RSH], in0=f1rep[h],
                                scalar=f2sb[:, cb * H + h:cb * H + h + 1],
                                in1=madj[cb], op0=Alu.add, op1=Alu.add)
                        _lrelu(hp * NCB + cb, u)
                        p = work.tile([128, RSH * 2], F16, tag="p")
                        nc.scalar.activation(out=p, in_=u, func=Act.Exp)
                        for i, h in enumerate((ha, hb)):
                            for j in range(2):
                                nc.tensor.matmul(
                                    ph1s[i][:, j * 512:(j + 1) * 512],
                                    lhsT=whaug[cb][:, h * 128:(h + 1) * 128],
                                    rhs=p[:, i * RSH + j * 512:i * RSH + (j + 1) * 512],
                                    start=(cb == 0), stop=(cb == NCB - 1))
                    # epilogue (both heads): normalize + ELU -> hT.
                    # DVE lanes are partition-fixed: recip stays on rows
                    # 64:128; a DMA shifts it down to rows 0:64.
                    for i, h in enumerate((ha, hb)):
                        ph1 = ph1s[i]
                        rs = ep.tile([128, RSH], F32, tag="rs")
                        nc.scalar.activation(out=rs[64:128, :], in_=ph1[64:128, :], func=Act.Ln)
                        nc.scalar.activation(out=rs[64:128, :], in_=rs[64:128, :], func=Act.Exp,
                                             scale=-1.0)
                        nc.sync.dma_start(out=rs[0:64, :], in_=rs[64:128, :])
                        hn = ep.tile([64, RSH], F16, tag="hn")
                        nc.vector.scalar_tensor_tensor(out=hn, in0=ph1[0:64, :], scalar=0.0,
                                                       in1=rs[0:64, :], op0=Alu.add, op1=Alu.mult)
                        m = ep.tile([64, RSH], F16, tag="m")
                        nc.vector.tensor_scalar(out=m, in0=hn, scalar1=0.0, scalar2=0.0,
                                                op0=Alu.min, op1=Alu.bypass)
                        q = ep.tile([64, RSH], F16, tag="q")
                        nc.scalar.activation(out=q, in_=m, func=Act.Exp)
                        if DBG and h == 0:
                            nc.sync.dma_start(out=dbg_names["dbg_rs0"][:, :], in_=rs)
                            nc.sync.dma_start(out=dbg_names["dbg_hn0"][:, :], in_=hn)
                            nc.sync.dma_start(out=dbg_names["dbg_q0"][:, :], in_=q)
                        if h % 2 == 0:
                            dst = hT[h // 2][0:64, :]
                            nc.vector.scalar_tensor_tensor(out=dst, in0=q, scalar=-1.0,
                                                           in1=hn, op0=Alu.add, op1=Alu.max)
                        else:
                            tmp = ep.tile([64, RSH], F16, tag="hodd")
                            nc.vector.scalar_tensor_tensor(out=tmp, in0=q, scalar=-1.0,
                                                           in1=hn, op0=Alu.add, op1=Alu.max)
                            nc.sync.dma_start(out=hT[h // 2][64:128, :], in_=tmp)
                        if DBG and h == 7:
                            pcopy = ep.tile([128, RSH], F32, tag="dbgph1")
                            nc.vector.tensor_copy(out=pcopy, in_=ph1)
                            nc.sync.dma_start(out=dbg_names["dbg_ph1_7"][:, :], in_=pcopy)

                # ---------------- Wh2 + exchange ----------------
                ccsb = res.tile([128, 8 * 65], F16, tag="ccsb")
                for nb in range(8):
                    pw2 = ps_main.tile([128, 65], F32, tag="wh2", bufs=2)
                    for k in range(4):
                        nc.tensor.matmul(pw2, lhsT=hT[k][:, nb * 128:(nb + 1) * 128],
                                         rhs=woaug[k], start=(k == 0), stop=(k == 3))
                    nc.vector.tensor_copy(out=ccsb[:, nb * 65:(nb + 1) * 65], in_=pw2)
                # ones columns 32:64 of every block
                ccsb_v = ccsb.rearrange("p (nb j) -> p nb j", j=65)
                nc.vector.memset(ccsb_v[:, :, 32:64], 1.0)
                nc.sync.dma_start(
                    out=cc_in[:, :].rearrange("(nb p) j -> p nb j", p=128),
                    in_=ccsb_v)
                nc.gpsimd.collective_compute(
                    "AllGather", Alu.bypass, replica_groups=GROUPS,
                    ins=[cc_in[:, :]], outs=[cc_out[:, :]])
                wh2 = [res.tile([128, 65], F16, tag=f"wh2_{cb}", name=f"wh2_{cb}") for cb in range(NCB)]
                cc_out_r = cc_out[:, :].rearrange("(cb p) j -> p cb j", p=128)
                for cb in range(NCB):
                    nc.sync.dma_start(out=wh2[cb], in_=cc_out_r[:, cb, :])
                if DBG:
                    nc.sync.dma_start(out=dbg_names["dbg_f1rep0"][:, :], in_=f1rep[0])
                    nc.sync.dma_start(out=dbg_names["dbg_f2sb"][:, :], in_=f2sb)
                    nc.sync.dma_start(out=dbg_names["dbg_whaug0"][:, :], in_=whaug[0])
                    nc.sync.dma_start(out=dbg_names["dbg_hT0"][:, :], in_=hT[0])
                    nc.sync.dma_start(out=dbg_names["dbg_wh2_0"][:, :], in_=wh2[0])
                    nc.sync.dma_start(out=dbg_names["dbg_ccsb"][:, :], in_=ccsb)

                # g1 replicated: [128, 1024]
                pg1 = ps_main.tile([128, RSH], F32, tag="h1")
                for j in range(2):
                    for k in range(4):
                        nc.tensor.matmul(pg1[:, j * 512:(j + 1) * 512],
                                         lhsT=wo1rep[k],
                                         rhs=hT[k][:, j * 512:(j + 1) * 512],
                                         start=(k == 0), stop=(k == 3))
                g1rep = res.tile([128, RSH], F16, tag="g1rep")
                nc.vector.tensor_copy(out=g1rep, in_=pg1)
                if DBG:
                    nc.sync.dma_start(out=dbg_names["dbg_g1rep"][:, :], in_=g1rep)

                # ---------------- layer 2 ----------------
                po = ps_main.tile([128, RSH], F32, tag="h1")
                for cbp in range(NCB // 2):
                    ca, cb2 = 2 * cbp, 2 * cbp + 1
                    u2 = work.tile([128, RSH * 2], F16, tag="u")
                    for i, cc in enumerate((ca, cb2)):
                        nc.vector.scalar_tensor_tensor(
                            out=u2[:, i * RSH:(i + 1) * RSH], in0=g1rep,
                            scalar=wh2[cc][:, 64:65],
                            in1=madj[cc], op0=Alu.add, op1=Alu.add)
                    _lrelu(64 + cbp, u2)
                    p2 = work.tile([128, RSH * 2], F16, tag="p")
                    nc.scalar.activation(out=p2, in_=u2, func=Act.Exp)
                    for i, cc in enumerate((ca, cb2)):
                        for j in range(2):
                            nc.tensor.matmul(
                                po[0:65, j * 512:(j + 1) * 512],
                                lhsT=wh2[cc],
                                rhs=p2[:, i * RSH + j * 512:i * RSH + (j + 1) * 512],
                                start=(cc == 0), stop=(cc == NCB - 1))
                rs2 = ep.tile([64, RSH], F32, tag="rs2")
                nc.scalar.activation(out=rs2[32:64, :], in_=po[32:64, :], func=Act.Ln)
                nc.scalar.activation(out=rs2[32:64, :], in_=rs2[32:64, :], func=Act.Exp,
                                     scale=-1.0)
                nc.sync.dma_start(out=rs2[0:32, :], in_=rs2[32:64, :])
                ov = ep.tile([32, RSH], F32, tag="ov")
                nc.vector.scalar_tensor_tensor(out=ov, in0=po[0:32, :], scalar=0.0,
                                               in1=rs2[0:32, :], op0=Alu.add, op1=Alu.mult)
                osb = ep.tile([32, RSH], F32, tag="osb")
                nc.vector.scalar_tensor_tensor(out=osb, in0=ov, scalar=OUT_SLOPE,
                                               in1=ov, op0=Alu.mult, op1=Alu.max)
                nc.sync.dma_start(out=out_e[:, :], in_=osb)
    from concourse.library_overlay import lower_extended_insts
    lower_extended_insts(nc)
    _legalize_multi_waits(nc)
    return nc


_NC = None


def _host_prep(x, adj, W, a1, a2, Wout, ao1, ao2):
    x = np.asarray(x, dtype=np.float32)
    adj = np.asarray(adj, dtype=np.float32)
    W = np.asarray(W, dtype=np.float32)
    a1 = np.asarray(a1, dtype=np.float32)
    a2 = np.asarray(a2, dtype=np.float32)
    Wout = np.asarray(Wout, dtype=np.float32)
    ao1 = np.asarray(ao1, dtype=np.float32)
    ao2 = np.asarray(ao2, dtype=np.float32)

    xT = np.ascontiguousarray(x.transpose(0, 2, 1))                # [B,128,N]
    madj = ((adj.transpose(0, 2, 1) - 1.0) * (-MASK)).astype(np.float16)  # 0 / -150, [B,N,N] as (c,r)
    wall = np.ascontiguousarray(W.transpose(1, 0, 2).reshape(F_IN, H * F_HID))
    wa1 = np.einsum('hfo,ho->fh', W, a1)                           # [128,H]
    wa2 = np.ascontiguousarray(np.einsum('hfo,ho->fh', W, a2))     # [128,H]
    w1rep = np.repeat(wa1, 128, axis=1)                            # [128,H*128]
    wo1 = Wout @ ao1                                               # [512]
    wo2 = Wout @ ao2
    woaug = np.zeros((H * F_HID, 65), np.float16)
    woaug[:, :C] = Wout.astype(np.float16)
    woaug[:, 64] = wo2.astype(np.float16)
    wo1rep = np.repeat(wo1[:, None], 128, axis=1).astype(np.float16)

    in_maps = []
    for c in range(8):
        b, s = c // 2, c % 2
        sl = slice(s * RSH, (s + 1) * RSH)
        in_maps.append({
            "xT": np.ascontiguousarray(xT[b]),
            "xtr": np.ascontiguousarray(xT[b][:, sl]),
            "madj": np.ascontiguousarray(madj[b][:, sl]),
            "wall": wall,
            "wa2": wa2,
            "w1rep": np.ascontiguousarray(w1rep),
            "woaug": woaug,
            "wo1rep": wo1rep,
        })
    return in_maps


def run(x, adj, W, a1, a2, Wout, ao1, ao2, trace=False, **trace_kw):
    global _NC
    if _NC is None:
        _NC = build_nc()
    in_maps = _host_prep(x, adj, W, a1, a2, Wout, ao1, ao2)
    r = run_bass_kernel_spmd(_NC, in_maps, list(range(8)), trace=trace, **trace_kw)
    out = np.empty((B, N, C), np.float32)
    for c in range(8):
        b, s = c // 2, c % 2
        out[b, s * RSH:(s + 1) * RSH, :] = r.results[c]["out"].T
    return out, r


def kernel(x, adj, W, a1, a2, Wout, ao1, ao2, batch_size=None):
    out, _ = run(x, adj, W, a1, a2, Wout, ao1, ao2)
    return out

